# revision 1
# baseline (speedup 1.0000x reference)
"""GAT layer on 8 TRN2 cores: dst-sharded, edge-gather + one-hot segment matmul.

Design:
  - Output nodes (dst) sharded contiguously across 8 cores (NPC nodes each).
  - Each core: phase 1 computes the FULL transformed-feature table
    h = x @ W (bf16) extended with per-node a_src logits, written to its DRAM
    (replicated compute; no collectives anywhere).
    Phase 1b computes a_dst logits for its own node range (SBUF resident).
  - Phase 2: edges of each 128-dst-node window, split into lo/hi src streams
    (int16 gather index limit), are gathered from the table (768B rows),
    scored (exp(leakyrelu(a_src + a_dst))), scaled, and segment-summed into
    PSUM via per-tile one-hot matmuls (fp8 one-hots shipped from host).
    Denominators ride in the same matmul (column 256:260); final normalize
    divides by them and adds bias.
"""
import sys
sys.path.insert(0, '/opt/trn_rl_repo')
import numpy as np
import ml_dtypes

import bass_rust as _br
import concourse.bacc as bacc
import concourse.mybir as mybir
import concourse.tile as tile
from concourse import bass_utils

BF16 = ml_dtypes.bfloat16
FP8 = ml_dtypes.float8_e4m3

C_IN = 128
C_OUT_TOT = 256   # HEADS * OUT_CH
HEADS = 4
HC = 64
NEG_SLOPE = 0.2
ROW = 384         # table row: 256 h + 4 a_src + 124 pad (bf16) = 768 B


def derive_cfg(N, n_cores, T_MAX):
    NPC = N // n_cores
    assert NPC * n_cores == N
    NW = (NPC + 127) // 128
    SPLIT = (N + 1) // 2
    assert SPLIT < 32768 and (N - SPLIT) < 32768
    return dict(N=N, n_cores=n_cores, NPC=NPC, NW=NW, SPLIT=SPLIT, T_MAX=T_MAX)


def host_prep(x, edge_index, W, att_src, att_dst, bias, n_cores=8):
    """Shard + schedule. Returns (cfg, in_maps)."""
    N = x.shape[0]
    E = edge_index.shape[1]
    src = np.concatenate([np.asarray(edge_index[0], np.int64),
                          np.arange(N, dtype=np.int64)]).astype(np.int32)
    dst = np.concatenate([np.asarray(edge_index[1], np.int64),
                          np.arange(N, dtype=np.int64)]).astype(np.int32)

    NPC = N // n_cores
    NW = (NPC + 127) // 128
    SPLIT = (N + 1) // 2

    # per (core, window, stream) edge lists
    core_of = dst // NPC
    lists = [[[None, None] for _ in range(NW)] for _ in range(n_cores)]
    order = np.argsort(dst, kind='stable')
    src_s, dst_s = src[order], dst[order]
    t_max = 1
    for c in range(n_cores):
        lo_c = np.searchsorted(dst_s, c * NPC, 'left')
        hi_c = np.searchsorted(dst_s, (c + 1) * NPC, 'left')
        sc, dc = src_s[lo_c:hi_c], dst_s[lo_c:hi_c]
        dl = dc - c * NPC
        for w in range(NW):
            m = (dl >= w * 128) & (dl < (w + 1) * 128)
            sw, dw = sc[m], dl[m] - w * 128
            for s in range(2):
                ms = (sw < SPLIT) if s == 0 else (sw >= SPLIT)
                ssw, sdw = sw[ms], dw[ms]
                lists[c][w][s] = (ssw, sdw)
                t_max = max(t_max, (len(ssw) + 127) // 128)

    T_MAX = t_max
    cfg = derive_cfg(N, n_cores, T_MAX)
    TS = T_MAX * 128          # slots per (w, s) call
    NCALL = NW * 2

    xT = np.ascontiguousarray(x.T).astype(BF16)            # [128, N]
    W_b = np.asarray(W, np.float32).astype(BF16)           # [128, 256]
    WT_b = np.ascontiguousarray(np.asarray(W).T).astype(BF16)  # [256, 128]
    att_flatT = np.zeros((C_OUT_TOT, 2 * HEADS), np.float32)
    for h in range(HEADS):
        att_flatT[h * HC:(h + 1) * HC, h] = np.asarray(att_src)[h]
        att_flatT[h * HC:(h + 1) * HC, HEADS + h] = np.asarray(att_dst)[h]
    att_flatT_b = att_flatT.astype(BF16)                   # [256, 8]
    bias_bc = np.broadcast_to(np.asarray(bias, np.float32), (128, C_OUT_TOT)).copy()

    in_maps = []
    for c in range(n_cores):
        idx16 = np.zeros((128, NCALL, TS // 16), np.int16)
        ohT = np.zeros((128, NCALL * TS), FP8)   # [e_lane, (call,tile)*128 p]
        ohF = np.zeros((128, NCALL * TS), FP8)   # [p_lane, (call,tile)*128 e]
        for w in range(NW):
            for s in range(2):
                k = w * 2 + s
                ssw, sdw = lists[c][w][s]
                n = len(ssw)
                idx = np.zeros(TS, np.int16)
                idx[:n] = (ssw - (SPLIT if s else 0)).astype(np.int16)
                wrapped = idx.reshape(TS // 16, 16).T
                idx16[:, k, :] = np.tile(wrapped, (8, 1))
                # one-hots
                e_pos = np.arange(n)
                lanes = e_pos % 128
                tiles = e_pos // 128
                ohT[lanes, k * TS + tiles * 128 + sdw] = 1.0
                ohF[sdw, k * TS + tiles * 128 + lanes] = 1.0
        in_maps.append({
            "xT": xT, "xT_own": np.ascontiguousarray(xT[:, c * NPC:(c + 1) * NPC]),
            "Wb": W_b, "WTb": WT_b, "attT": att_flatT_b, "bias_bc": bias_bc,
            "idx16": idx16, "ohT": ohT, "ohF": ohF,
        })
    return cfg, in_maps


def build_program(cfg, dbg=False):
    N, NPC, NW, SPLIT, T_MAX = (cfg[k] for k in ("N", "NPC", "NW", "SPLIT", "T_MAX"))
    n_cores = cfg["n_cores"]
    TS = T_MAX * 128
    NCALL = NW * 2
    NT = (N + 127) // 128          # node tiles for table build
    dt = mybir.dt

    nc = bacc.Bacc("TRN2", target_bir_lowering=False, debug=False,
                   num_devices=n_cores)
    t_xT = nc.dram_tensor("xT", (128, N), dt.bfloat16, kind="ExternalInput")
    t_xT_own = nc.dram_tensor("xT_own", (128, NPC), dt.bfloat16, kind="ExternalInput")
    t_Wb = nc.dram_tensor("Wb", (C_IN, C_OUT_TOT), dt.bfloat16, kind="ExternalInput")
    t_WTb = nc.dram_tensor("WTb", (C_OUT_TOT, C_IN), dt.bfloat16, kind="ExternalInput")
    t_attT = nc.dram_tensor("attT", (C_OUT_TOT, 2 * HEADS), dt.bfloat16, kind="ExternalInput")
    t_bias = nc.dram_tensor("bias_bc", (128, C_OUT_TOT), dt.float32, kind="ExternalInput")
    t_idx = nc.dram_tensor("idx16", (128, NCALL, TS // 16), dt.int16, kind="ExternalInput")
    t_ohT = nc.dram_tensor("ohT", (128, NCALL * TS), dt.float8e4, kind="ExternalInput")
    t_ohF = nc.dram_tensor("ohF", (128, NCALL * TS), dt.float8e4, kind="ExternalInput")
    t_htab = nc.dram_tensor("htab", (N, ROW), dt.bfloat16,
                            kind="ExternalOutput" if dbg else "Internal")
    t_out = nc.dram_tensor("out", (NPC, C_OUT_TOT), dt.float32, kind="ExternalOutput")
    if dbg:
        t_dbg_adst = nc.dram_tensor("dbg_adst", (128, NW, HEADS), dt.float32, kind="ExternalOutput")
        t_dbg_e2 = nc.dram_tensor("dbg_e2", (128, 2, NW, HEADS * 8), dt.float32, kind="ExternalOutput")
        t_dbg_msg = nc.dram_tensor("dbg_msg", (128, 2, 264 * 8), dt.float32, kind="ExternalOutput")
        t_dbg_psw = nc.dram_tensor("dbg_psw", (128, NW, 260), dt.float32, kind="ExternalOutput")

    with tile.TileContext(nc) as tc:
        with tc.tile_pool(name="const", bufs=1) as cpool:
            W_sb = cpool.tile([C_IN, C_OUT_TOT], dt.bfloat16)
            nc.sync.dma_start(out=W_sb, in_=t_Wb.ap())
            bias_sb = cpool.tile([128, C_OUT_TOT], dt.float32)
            nc.sync.dma_start(out=bias_sb, in_=t_bias.ap())
            idx_sb = cpool.tile([128, NCALL, TS // 16], dt.int16)
            nc.sync.dma_start(out=idx_sb, in_=t_idx.ap())
            adst_sb = cpool.tile([128, NW, HEADS], dt.bfloat16)
            nc.vector.memset(adst_sb, 0)

            # w_att = W @ att_flatT  via  (WT_blk)^T @ attT_blk, accumulated
            with tc.tile_pool(name="watt_ps", bufs=1, space="PSUM") as wpp, \
                 tc.tile_pool(name="watt_sb", bufs=1) as wsp:
                ps_watt = wpp.tile([C_IN, 2 * HEADS], dt.float32)
                wt0 = wsp.tile([128, C_IN], dt.bfloat16)
                wt1 = wsp.tile([128, C_IN], dt.bfloat16)
                at0 = wsp.tile([128, 2 * HEADS], dt.bfloat16)
                at1 = wsp.tile([128, 2 * HEADS], dt.bfloat16)
                nc.sync.dma_start(out=wt0, in_=t_WTb.ap()[0:128, :])
                nc.sync.dma_start(out=wt1, in_=t_WTb.ap()[128:256, :])
                nc.sync.dma_start(out=at0, in_=t_attT.ap()[0:128, :])
                nc.sync.dma_start(out=at1, in_=t_attT.ap()[128:256, :])
                nc.tensor.matmul(out=ps_watt, lhsT=wt0, rhs=at0, start=True, stop=False)
                nc.tensor.matmul(out=ps_watt, lhsT=wt1, rhs=at1, start=False, stop=True)
                watt_sb = cpool.tile([C_IN, 2 * HEADS], dt.bfloat16)
                nc.vector.tensor_copy(out=watt_sb, in_=ps_watt)

            # ---------- phase 1: h table ----------
            htab_writes = []
            CHUNK = 12544  # 98 node-tiles per chunk (25 KB/partition bf16)
            with tc.tile_pool(name="p1x", bufs=2) as p1x, \
                 tc.tile_pool(name="p1h", bufs=3) as p1h, \
                 tc.tile_pool(name="p1ps", bufs=4, space="PSUM") as p1ps:
                for ci in range(0, N, CHUNK):
                    cw = min(CHUNK, N - ci)
                    xc = p1x.tile([128, CHUNK], dt.bfloat16, tag="xc")
                    nc.sync.dma_start(out=xc[:, 0:cw], in_=t_xT.ap()[:, ci:ci + cw])
                    for nt0 in range(0, cw, 128):
                        nn = min(128, cw - nt0)
                        ps_h = p1ps.tile([128, C_OUT_TOT], dt.float32, tag="ps_h")
                        ps_l = p1ps.tile([128, 2 * HEADS], dt.float32, tag="ps_l")
                        lhsT = xc[:, nt0:nt0 + nn]
                        nc.tensor.matmul(out=ps_h[0:nn, :], lhsT=lhsT, rhs=W_sb,
                                         start=True, stop=True)
                        nc.tensor.matmul(out=ps_l[0:nn, :], lhsT=lhsT, rhs=watt_sb,
                                         start=True, stop=True)
                        hsb = p1h.tile([128, ROW], dt.bfloat16, tag="hsb")
                        nc.scalar.copy(out=hsb[0:nn, 0:C_OUT_TOT], in_=ps_h[0:nn, :])
                        nc.vector.tensor_copy(out=hsb[0:nn, C_OUT_TOT:C_OUT_TOT + HEADS],
                                              in_=ps_l[0:nn, 0:HEADS])
                        nc.vector.memset(hsb[0:nn, C_OUT_TOT + HEADS:ROW], 0)
                        n0 = ci + nt0
                        htab_writes.append(
                            nc.sync.dma_start(out=t_htab.ap()[n0:n0 + nn, :],
                                              in_=hsb[0:nn, :]))

            # ---------- phase 1b: a_dst for own range ----------
            with tc.tile_pool(name="p1bx", bufs=2) as p1bx, \
                 tc.tile_pool(name="p1bps", bufs=2, space="PSUM") as p1bps:
                for w in range(NW):
                    nn = min(128, NPC - w * 128)
                    xo = p1bx.tile([128, 128], dt.bfloat16, tag="xo")
                    nc.sync.dma_start(out=xo[:, 0:nn],
                                      in_=t_xT_own.ap()[:, w * 128:w * 128 + nn])
                    ps_l2 = p1bps.tile([128, 2 * HEADS], dt.float32, tag="ps_l2")
                    nc.tensor.matmul(out=ps_l2[0:nn, :], lhsT=xo[:, 0:nn], rhs=watt_sb,
                                     start=True, stop=True)
                    nc.vector.tensor_copy(out=adst_sb[0:nn, w, :],
                                          in_=ps_l2[0:nn, HEADS:2 * HEADS])

            if dbg:
                dbg_adst_f = cpool.tile([128, NW, HEADS], dt.float32)
                nc.vector.tensor_copy(out=dbg_adst_f, in_=adst_sb)
                nc.sync.dma_start(out=t_dbg_adst.ap(), in_=dbg_adst_f)

            # ---------- phase 2 ----------
            # RAW fence: Tile does not track deps through DRAM tensors, so
            # gathers must explicitly wait for all htab writes.
            fence = nc.sync.nop(hint="htab_fence", nofuse=True)
            for _wi in htab_writes:
                _br.add_dep_helper(fence.ins, _wi.ins, reason="htab RAW fence")
            ap_lo = t_htab.ap()[0:SPLIT, :]
            ap_hi = t_htab.ap()[SPLIT:N, :]
            with tc.tile_pool(name="p2g", bufs=4) as p2g, \
                 tc.tile_pool(name="p2o", bufs=4) as p2o, \
                 tc.tile_pool(name="p2m", bufs=2) as p2m, \
                 tc.tile_pool(name="p2s", bufs=3) as p2s, \
                 tc.tile_pool(name="p2ps", bufs=2, space="PSUM") as p2ps, \
                 tc.tile_pool(name="p2pse", bufs=2, space="PSUM") as p2pse:
                for w in range(NW):
                    nn = min(128, NPC - w * 128)
                    gb = [None, None]
                    for s in range(2):
                        k = w * 2 + s
                        gb[s] = p2g.tile([128, T_MAX, ROW], dt.bfloat16, tag=f"gb{s}", name=f"gb{s}")
                        _g = nc.gpsimd.dma_gather(
                            out_ap=gb[s], in_ap=(ap_lo if s == 0 else ap_hi),
                            idxs_ap=idx_sb[:, k, :],
                            num_idxs=TS, num_idxs_reg=TS, elem_size=ROW,
                            single_packet=False,
                        )
                        _br.add_dep_helper(_g.ins, fence.ins, reason="htab RAW fence")
                    ohT_b = p2o.tile([128, 2 * TS], dt.float8e4, tag="ohT")
                    ohF_b = p2o.tile([128, 2 * TS], dt.float8e4, tag="ohF")
                    nc.sync.dma_start(out=ohT_b, in_=t_ohT.ap()[:, 2 * w * TS:(2 * w + 2) * TS])
                    nc.sync.dma_start(out=ohF_b, in_=t_ohF.ap()[:, 2 * w * TS:(2 * w + 2) * TS])

                    # a_dst per edge: [128e, 4] per tile -> ps_adst[:, s, t, :]
                    ps_adst = p2pse.tile([128, 2, T_MAX, HEADS], dt.float32, tag="ps_adst")
                    for s in range(2):
                        for t in range(T_MAX):
                            nc.tensor.matmul(
                                out=ps_adst[:, s, t, :],
                                lhsT=ohF_b[:, (s * T_MAX + t) * 128:(s * T_MAX + t + 1) * 128],
                                rhs=adst_sb[:, w, :],
                                start=True, stop=True)

                    # e = exp(lrelu(a_src + a_dst)) -> msg[:, :, :, 256:260]
                    msg = p2m.tile([128, 2, T_MAX, 264], dt.bfloat16, tag="msg")
                    e_tmp = p2s.tile([128, 2, T_MAX, HEADS], dt.float32, tag="e_tmp")
                    for s in range(2):
                        nc.vector.tensor_tensor(
                            out=e_tmp[:, s, :, :], in0=ps_adst[:, s, :, :],
                            in1=gb[s][:, :, C_OUT_TOT:C_OUT_TOT + HEADS],
                            op=mybir.AluOpType.add)
                    e2 = p2s.tile([128, 2, T_MAX, HEADS], dt.float32, tag="e2")
                    nc.scalar.activation(out=e2, in_=e_tmp,
                                         func=mybir.ActivationFunctionType.Prelu,
                                         alpha=NEG_SLOPE)
                    nc.scalar.activation(out=msg[:, :, :, 256:260], in_=e2,
                                         func=mybir.ActivationFunctionType.Exp)

                    # msg = h * ex (broadcast per head)
                    for s in range(2):
                        for t in range(T_MAX):
                            exb = msg[:, s, t, 256:260].unsqueeze(2).broadcast_to(
                                [128, HEADS, HC])
                            nc.vector.tensor_tensor(
                                out=msg[:, s, t, 0:C_OUT_TOT].rearrange(
                                    "p (h c) -> p h c", h=HEADS),
                                in0=gb[s][:, t, 0:C_OUT_TOT].rearrange(
                                    "p (h c) -> p h c", h=HEADS),
                                in1=exb, op=mybir.AluOpType.mult)

                    # aggregate: psum[p, 0:256] += msg, psum[p, 256:260] += ex
                    ps_win = p2ps.tile([128, 260], dt.float32, tag="ps_win")
                    for s in range(2):
                        for t in range(T_MAX):
                            first = (s == 0 and t == 0)
                            last = (s == 1 and t == T_MAX - 1)
                            nc.tensor.matmul(
                                out=ps_win,
                                lhsT=ohT_b[:, (s * T_MAX + t) * 128:(s * T_MAX + t + 1) * 128],
                                rhs=msg[:, s, t, 0:260],
                                start=first, stop=last)

                    if dbg:
                        TT = min(T_MAX, 8)
                        de2 = p2s.tile([128, 2, T_MAX, HEADS], dt.float32, tag="de2", name="de2")
                        nc.vector.tensor_copy(out=de2, in_=e2)
                        nc.sync.dma_start(out=t_dbg_e2.ap()[:, :, w, 0:HEADS * TT],
                                          in_=de2[:, :, 0:TT, :].rearrange("p s t h -> p s (t h)"))
                        if w == 0:
                            dmsg = p2s.tile([128, 2, 8, 264], dt.float32, tag="dmsg", name="dmsg")
                            nc.vector.tensor_copy(out=dmsg[:, :, 0:TT, :], in_=msg[:, :, 0:TT, :])
                            nc.sync.dma_start(out=t_dbg_msg.ap(),
                                              in_=dmsg.rearrange("p s t f -> p s (t f)"))
                        dpsw = p2s.tile([128, 260], dt.float32, tag="dpsw", name="dpsw")
                        nc.vector.tensor_copy(out=dpsw, in_=ps_win)
                        nc.sync.dma_start(out=t_dbg_psw.ap()[:, w, :], in_=dpsw)

                    # normalize + bias
                    rcp = p2s.tile([128, HEADS], dt.float32, tag="rcp")
                    nc.vector.reciprocal(out=rcp, in_=ps_win[:, 256:260])
                    osb = p2s.tile([128, C_OUT_TOT], dt.float32, tag="osb")
                    for h in range(HEADS):
                        nc.vector.tensor_scalar(
                            out=osb[:, h * HC:(h + 1) * HC],
                            in0=ps_win[:, h * HC:(h + 1) * HC],
                            scalar1=rcp[:, h:h + 1], scalar2=None,
                            op0=mybir.AluOpType.mult)
                    nc.vector.tensor_tensor(out=osb, in0=osb, in1=bias_sb,
                                            op=mybir.AluOpType.add)
                    nc.sync.dma_start(out=t_out.ap()[w * 128:w * 128 + nn, :],
                                      in_=osb[0:nn, :])

    nc.finalize()
    return nc


def register_ntff_hook():
    import types
    import antenv
    if getattr(antenv, 'axon_hooks', None) is not None:
        return
    mod = types.ModuleType('antenv.axon_hooks')
    _hook = [None]
    mod.set_axon_ntff_profile_hook = lambda h: _hook.__setitem__(0, h)
    mod.get_axon_ntff_profile_hook = lambda: _hook[0]
    sys.modules['antenv.axon_hooks'] = mod
    antenv.axon_hooks = mod
    try:
        from trn_agent_boot.trn_boot import _ntff_profile_via_ctypes
        mod.set_axon_ntff_profile_hook(
            _ntff_profile_via_ctypes('/opt/axon/libaxon_pjrt.so'))
    except Exception:
        pass


def run(x, edge_index, W, att_src, att_dst, bias, n_cores=8, trace=False):
    cfg, in_maps = host_prep(x, edge_index, W, att_src, att_dst, bias, n_cores)
    nc = build_program(cfg)
    if trace:
        register_ntff_hook()
    r = bass_utils.run_bass_kernel_spmd(nc, in_maps,
                                        core_ids=list(range(n_cores)),
                                        trace=trace)
    out = np.concatenate([r.results[c]["out"] for c in range(n_cores)], axis=0)
    return out, r


# ----------------------------------------------------------------------------
# Self-contained harness entry point: full inputs in, full output out.
# ----------------------------------------------------------------------------
import os as _os


def kernel(x, edge_index, W, att_src, att_dst, bias):
    x = np.asarray(x, np.float32)
    edge_index = np.asarray(edge_index)
    W = np.asarray(W, np.float32)
    att_src = np.asarray(att_src, np.float32)
    att_dst = np.asarray(att_dst, np.float32)
    bias = np.asarray(bias, np.float32)
    trace = _os.environ.get("GAT_TRACE", "0") == "1"
    out, r = run(x, edge_index, W, att_src, att_dst, bias, n_cores=8, trace=trace)
    if trace and r.exec_time_ns is not None:
        print(f"HW exec time: {r.exec_time_ns} ns")
    return np.ascontiguousarray(out.astype(np.float32))



# revision 2
# speedup vs baseline: 1.3111x; 1.3111x over previous
"""GAT layer on 8 TRN2 cores — V3: gather x-rows (256 B) in transpose mode,
recompute h per edge on TensorE; no DRAM h-table, no phase-1 critical path.

Design:
  - dst windows (128 output nodes each, 391 global) bin-packed onto 8 cores
    by per-window tile count so the SPMD program shape matches all cores.
  - Per edge, gather the 256 B bf16 x-row of its src node from a host-staged
    [N, 128] table via gpsimd dma_gather(transpose=True): output lands as
    [128 c, slots e] — directly the lhsT layout for the h matmul.
  - int16 gather indices use a mid-table base (row 32768) so negative
    indices cover rows [0, 32768) and positives [32768, 50000): one stream.
  - Per 128-edge tile: he = xe_tile^T @ Wext ([128 e, 264] PSUM: 256 h,
    4 a_src-logit, 4 a_dst-logit cols); a_dst per edge via one-hot ohF
    matmul vs per-window a_dst table; e = exp(leakyrelu(a_src + a_dst));
    msg = h * e; one-hot ohT matmul segment-sums msg + denominators into
    PSUM; normalize + bias.
  - Critical path = gather descriptor-gen on GpSimd (~8.3 ns/idx); all
    PE/Vector/Scalar work hides under it. Gathers start at t~=0 (no table
    build dependency).
"""
import sys
sys.path.insert(0, '/opt/trn_rl_repo')
import numpy as np
import ml_dtypes

import bass_rust as _br
import concourse.bacc as bacc
import concourse.mybir as mybir
import concourse.tile as tile
from concourse import bass_utils

BF16 = ml_dtypes.bfloat16
FP8 = ml_dtypes.float8_e4m3

C_IN = 128
C_OUT_TOT = 256   # HEADS * OUT_CH
HEADS = 4
HC = 64
NEG_SLOPE = 0.2
MID = 32768       # gather index base row
G = 4             # windows per gather call


def host_prep(x, edge_index, W, att_src, att_dst, bias, n_cores=8):
    """Shard + schedule. Returns (cfg, in_maps, win_of_slot per core)."""
    N = x.shape[0]
    E = edge_index.shape[1]
    src = np.concatenate([np.asarray(edge_index[0], np.int64),
                          np.arange(N, dtype=np.int64)]).astype(np.int32)
    dst = np.concatenate([np.asarray(edge_index[1], np.int64),
                          np.arange(N, dtype=np.int64)]).astype(np.int32)

    NWG = (N + 127) // 128                     # global windows
    NW = (NWG + n_cores - 1) // n_cores        # window slots per core

    # group edges by global window, sorted by dst
    order = np.argsort(dst, kind='stable')
    src_s, dst_s = src[order], dst[order]
    w_lo = np.searchsorted(dst_s, np.arange(NWG) * 128, 'left')
    w_hi = np.searchsorted(dst_s, (np.arange(NWG) + 1) * 128, 'left')
    w_cnt = w_hi - w_lo
    w_tiles = np.maximum((w_cnt + 127) // 128, 1)

    # bin-pack windows onto cores: biggest-first to least-loaded core
    core_wins = [[] for _ in range(n_cores)]
    core_load = np.zeros(n_cores, np.int64)
    for wid in np.argsort(-w_tiles, kind='stable'):
        c = int(np.argmin(core_load))
        core_wins[c].append(int(wid))
        core_load[c] += w_tiles[wid]
    for c in range(n_cores):                   # pad with -1 dummy windows
        core_wins[c] += [-1] * (NW - len(core_wins[c]))

    # per-slot tile count: max across cores (SPMD shape)
    T_slot = np.ones(NW, np.int64)
    for c in range(n_cores):
        # sort slots by tile count desc so maxima align
        ws = core_wins[c]
        real = sorted([w for w in ws if w >= 0], key=lambda w: -w_tiles[w])
        core_wins[c] = real + [-1] * (NW - len(real))
        for k, w in enumerate(core_wins[c]):
            if w >= 0:
                T_slot[k] = max(T_slot[k], w_tiles[w])

    NCALL = (NW + G - 1) // G
    call_slots = []
    for k in range(NCALL):
        ts = int(T_slot[k * G:(k + 1) * G].sum()) * 128
        call_slots.append(ts)
    TOT = sum(call_slots)

    cfg = dict(N=N, n_cores=n_cores, NW=NW, NCALL=NCALL,
               T_slot=[int(t) for t in T_slot], call_slots=call_slots,
               TOT=TOT)

    xT16 = np.ascontiguousarray(np.asarray(x, np.float32)).astype(BF16)  # [N,128]
    W_b = np.asarray(W, np.float32).astype(BF16)           # [128, 256]
    WT_b = np.ascontiguousarray(np.asarray(W).T).astype(BF16)  # [256, 128]
    att_flatT = np.zeros((C_OUT_TOT, 2 * HEADS), np.float32)
    for h in range(HEADS):
        att_flatT[h * HC:(h + 1) * HC, h] = np.asarray(att_src)[h]
        att_flatT[h * HC:(h + 1) * HC, HEADS + h] = np.asarray(att_dst)[h]
    att_flatT_b = att_flatT.astype(BF16)                   # [256, 8]
    bias_bc = np.broadcast_to(np.asarray(bias, np.float32), (128, C_OUT_TOT)).copy()
    xTfull = xT16.T                                        # [128, N] view

    in_maps = []
    for c in range(n_cores):
        idx = np.zeros(TOT, np.int16)
        ohT = np.zeros((128, TOT), FP8)
        ohF = np.zeros((128, TOT), FP8)
        xoT = np.zeros((128, NW * 128), BF16)
        col = 0
        for k, wid in enumerate(core_wins[c]):
            ts = int(T_slot[k]) * 128
            if wid >= 0:
                nn = min(128, N - wid * 128)
                xoT[:, k * 128:k * 128 + nn] = xTfull[:, wid * 128:wid * 128 + nn]
                sw = src_s[w_lo[wid]:w_hi[wid]]
                dw = dst_s[w_lo[wid]:w_hi[wid]] - wid * 128
                n = len(sw)
                assert n <= ts
                idx[col:col + n] = (sw - MID).astype(np.int16)
                # trailing-negative guard: last element of window block
                if n == ts and n > 0 and sw[n - 1] < MID:
                    pos = np.nonzero(sw >= MID)[0]
                    assert len(pos), "window with all-src<MID and full tiles"
                    p = pos[-1]
                    idx[col + p], idx[col + n - 1] = idx[col + n - 1], idx[col + p]
                    sw = sw.copy()
                    sw[p], sw[n - 1] = sw[n - 1], sw[p]
                    dw = dw.copy()
                    dw[p], dw[n - 1] = dw[n - 1], dw[p]
                e_pos = np.arange(n)
                lanes = e_pos % 128
                tiles = e_pos // 128
                ohT[lanes, col + tiles * 128 + dw] = 1.0
                ohF[dw, col + tiles * 128 + lanes] = 1.0
            col += ts
        assert col == TOT
        wrapped = idx.reshape(TOT // 16, 16).T             # [16, TOT/16]
        idx16 = np.tile(wrapped, (8, 1)).copy()            # [128, TOT/16]
        in_maps.append({
            "xtab": xT16, "xoT": xoT,
            "Wb": W_b, "WTb": WT_b, "attT": att_flatT_b, "bias_bc": bias_bc,
            "idx16": idx16, "ohT": ohT, "ohF": ohF,
        })
    return cfg, in_maps, core_wins


def build_program(cfg):
    N, NW, NCALL, TOT = (cfg[k] for k in ("N", "NW", "NCALL", "TOT"))
    T_slot, call_slots = cfg["T_slot"], cfg["call_slots"]
    n_cores = cfg["n_cores"]
    MAXSLOTS = max(call_slots)
    dt = mybir.dt

    nc = bacc.Bacc("TRN2", target_bir_lowering=False, debug=False,
                   num_devices=n_cores)
    t_xtab = nc.dram_tensor("xtab", (N, C_IN), dt.bfloat16, kind="ExternalInput")
    t_xoT = nc.dram_tensor("xoT", (C_IN, NW * 128), dt.bfloat16, kind="ExternalInput")
    t_Wb = nc.dram_tensor("Wb", (C_IN, C_OUT_TOT), dt.bfloat16, kind="ExternalInput")
    t_WTb = nc.dram_tensor("WTb", (C_OUT_TOT, C_IN), dt.bfloat16, kind="ExternalInput")
    t_attT = nc.dram_tensor("attT", (C_OUT_TOT, 2 * HEADS), dt.bfloat16, kind="ExternalInput")
    t_bias = nc.dram_tensor("bias_bc", (128, C_OUT_TOT), dt.float32, kind="ExternalInput")
    t_idx = nc.dram_tensor("idx16", (128, TOT // 16), dt.int16, kind="ExternalInput")
    t_ohT = nc.dram_tensor("ohT", (128, TOT), dt.float8e4, kind="ExternalInput")
    t_ohF = nc.dram_tensor("ohF", (128, TOT), dt.float8e4, kind="ExternalInput")
    t_out = nc.dram_tensor("out", (NW * 128, C_OUT_TOT), dt.float32, kind="ExternalOutput")

    ap_mid = t_xtab.ap()[MID:N, :]

    with tile.TileContext(nc) as tc:
        with tc.tile_pool(name="const", bufs=1) as cpool:
            idx_sb = cpool.tile([128, TOT // 16], dt.int16)
            nc.sync.dma_start(out=idx_sb, in_=t_idx.ap())
            Wext_sb = cpool.tile([C_IN, C_OUT_TOT + 2 * HEADS], dt.bfloat16)
            nc.sync.dma_start(out=Wext_sb[:, 0:C_OUT_TOT], in_=t_Wb.ap())
            bias_sb = cpool.tile([128, C_OUT_TOT], dt.float32)
            nc.sync.dma_start(out=bias_sb, in_=t_bias.ap())
            adst_sb = cpool.tile([128, NW, HEADS], dt.bfloat16)

            # w_att = W @ att_flatT -> Wext cols 256:264
            with tc.tile_pool(name="watt_ps", bufs=1, space="PSUM") as wpp, \
                 tc.tile_pool(name="watt_sb", bufs=1) as wsp:
                ps_watt = wpp.tile([C_IN, 2 * HEADS], dt.float32)
                wt0 = wsp.tile([128, C_IN], dt.bfloat16)
                wt1 = wsp.tile([128, C_IN], dt.bfloat16)
                at0 = wsp.tile([128, 2 * HEADS], dt.bfloat16)
                at1 = wsp.tile([128, 2 * HEADS], dt.bfloat16)
                nc.sync.dma_start(out=wt0, in_=t_WTb.ap()[0:128, :])
                nc.sync.dma_start(out=wt1, in_=t_WTb.ap()[128:256, :])
                nc.sync.dma_start(out=at0, in_=t_attT.ap()[0:128, :])
                nc.sync.dma_start(out=at1, in_=t_attT.ap()[128:256, :])
                nc.tensor.matmul(out=ps_watt, lhsT=wt0, rhs=at0, start=True, stop=False)
                nc.tensor.matmul(out=ps_watt, lhsT=wt1, rhs=at1, start=False, stop=True)
                nc.vector.tensor_copy(out=Wext_sb[:, C_OUT_TOT:C_OUT_TOT + 2 * HEADS],
                                      in_=ps_watt)

            # a_dst for own windows
            with tc.tile_pool(name="adx", bufs=2) as adx, \
                 tc.tile_pool(name="adps", bufs=2, space="PSUM") as adps:
                for w in range(NW):
                    xo = adx.tile([128, 128], dt.bfloat16, tag="xo")
                    nc.sync.dma_start(out=xo, in_=t_xoT.ap()[:, w * 128:(w + 1) * 128])
                    ps_a = adps.tile([128, HEADS], dt.float32, tag="ps_a")
                    nc.tensor.matmul(out=ps_a, lhsT=xo,
                                     rhs=Wext_sb[:, C_OUT_TOT + HEADS:C_OUT_TOT + 2 * HEADS],
                                     start=True, stop=True)
                    nc.vector.tensor_copy(out=adst_sb[:, w, :], in_=ps_a)

            # ---------- main: gather + per-tile pipeline ----------
            with tc.tile_pool(name="xep", bufs=2) as xep, \
                 tc.tile_pool(name="ohp", bufs=2) as ohp, \
                 tc.tile_pool(name="msgp", bufs=3) as msgp, \
                 tc.tile_pool(name="sp", bufs=4) as sp, \
                 tc.tile_pool(name="hps", bufs=3, space="PSUM") as hps, \
                 tc.tile_pool(name="wps", bufs=2, space="PSUM") as wps, \
                 tc.tile_pool(name="aps", bufs=2, space="PSUM") as aps:
                col0 = 0          # global slot offset of current call
                for k in range(NCALL):
                    cs = call_slots[k]
                    xe = xep.tile([128, 1, MAXSLOTS], dt.bfloat16, tag="xe")
                    nc.gpsimd.dma_gather(
                        out_ap=xe[:, :, 0:cs], in_ap=ap_mid,
                        idxs_ap=idx_sb[:, col0 // 16:(col0 + cs) // 16],
                        num_idxs=cs, num_idxs_reg=cs, elem_size=C_IN,
                        transpose=True, single_packet=False,
                    )
                    ohT_b = ohp.tile([128, MAXSLOTS], dt.float8e4, tag="ohT")
                    ohF_b = ohp.tile([128, MAXSLOTS], dt.float8e4, tag="ohF")
                    nc.sync.dma_start(out=ohT_b[:, 0:cs],
                                      in_=t_ohT.ap()[:, col0:col0 + cs])
                    nc.sync.dma_start(out=ohF_b[:, 0:cs],
                                      in_=t_ohF.ap()[:, col0:col0 + cs])

                    ccol = 0      # call-local slot offset
                    for w in range(k * G, min((k + 1) * G, NW)):
                        T = T_slot[w]
                        # a_dst per edge for the whole window
                        ps_adw = aps.tile([128, T, HEADS], dt.float32, tag="ps_adw",
                                          name=f"ps_adw{T}")
                        for t in range(T):
                            nc.tensor.matmul(
                                out=ps_adw[:, t, :],
                                lhsT=ohF_b[:, ccol + t * 128:ccol + (t + 1) * 128],
                                rhs=adst_sb[:, w, :], start=True, stop=True)
                        adw = sp.tile([128, T, HEADS], dt.float32, tag="adw",
                                      name=f"adw{T}")
                        nc.vector.tensor_copy(out=adw, in_=ps_adw)

                        msg = msgp.tile([128, T, 264], dt.bfloat16, tag="msg",
                                        name=f"msg{T}")
                        for t in range(T):
                            ps_he = hps.tile([128, 264], dt.float32, tag="ps_he")
                            nc.tensor.matmul(
                                out=ps_he,
                                lhsT=xe[:, 0, ccol + t * 128:ccol + (t + 1) * 128],
                                rhs=Wext_sb, start=True, stop=True)
                            eraw = sp.tile([128, HEADS], dt.float32, tag="eraw")
                            nc.vector.tensor_tensor(
                                out=eraw, in0=ps_he[:, 256:260], in1=adw[:, t, :],
                                op=mybir.AluOpType.add)
                            e2 = sp.tile([128, HEADS], dt.float32, tag="e2")
                            nc.scalar.activation(out=e2, in_=eraw,
                                                 func=mybir.ActivationFunctionType.Prelu,
                                                 alpha=NEG_SLOPE)
                            nc.scalar.activation(out=msg[:, t, 256:260], in_=e2,
                                                 func=mybir.ActivationFunctionType.Exp)
                            exb = msg[:, t, 256:260].unsqueeze(2).broadcast_to(
                                [128, HEADS, HC])
                            nc.vector.tensor_tensor(
                                out=msg[:, t, 0:C_OUT_TOT].rearrange(
                                    "p (h c) -> p h c", h=HEADS),
                                in0=ps_he[:, 0:C_OUT_TOT].rearrange(
                                    "p (h c) -> p h c", h=HEADS),
                                in1=exb, op=mybir.AluOpType.mult)

                        ps_win = wps.tile([128, 260], dt.float32, tag="ps_win")
                        for t in range(T):
                            nc.tensor.matmul(
                                out=ps_win,
                                lhsT=ohT_b[:, ccol + t * 128:ccol + (t + 1) * 128],
                                rhs=msg[:, t, 0:260],
                                start=(t == 0), stop=(t == T - 1))

                        rcp = sp.tile([128, HEADS], dt.float32, tag="rcp")
                        nc.vector.reciprocal(out=rcp, in_=ps_win[:, 256:260])
                        osb = sp.tile([128, C_OUT_TOT], dt.float32, tag="osb")
                        for h in range(HEADS):
                            nc.vector.tensor_scalar(
                                out=osb[:, h * HC:(h + 1) * HC],
                                in0=ps_win[:, h * HC:(h + 1) * HC],
                                scalar1=rcp[:, h:h + 1], scalar2=None,
                                op0=mybir.AluOpType.mult)
                        nc.vector.tensor_tensor(out=osb, in0=osb, in1=bias_sb,
                                                op=mybir.AluOpType.add)
                        nc.sync.dma_start(out=t_out.ap()[w * 128:(w + 1) * 128, :],
                                          in_=osb)
                        ccol += T * 128
                    col0 += cs

    nc.finalize()
    return nc


def register_ntff_hook():
    import types
    import antenv
    if getattr(antenv, 'axon_hooks', None) is not None:
        return
    mod = types.ModuleType('antenv.axon_hooks')
    _hook = [None]
    mod.set_axon_ntff_profile_hook = lambda h: _hook.__setitem__(0, h)
    mod.get_axon_ntff_profile_hook = lambda: _hook[0]
    sys.modules['antenv.axon_hooks'] = mod
    antenv.axon_hooks = mod
    try:
        from trn_agent_boot.trn_boot import _ntff_profile_via_ctypes
        mod.set_axon_ntff_profile_hook(
            _ntff_profile_via_ctypes('/opt/axon/libaxon_pjrt.so'))
    except Exception:
        pass


def run(x, edge_index, W, att_src, att_dst, bias, n_cores=8, trace=False):
    cfg, in_maps, core_wins = host_prep(x, edge_index, W, att_src, att_dst,
                                        bias, n_cores)
    nc = build_program(cfg)
    if trace:
        register_ntff_hook()
    r = bass_utils.run_bass_kernel_spmd(nc, in_maps,
                                        core_ids=list(range(n_cores)),
                                        trace=trace)
    N = cfg["N"]
    out = np.empty((N, C_OUT_TOT), np.float32)
    for c in range(n_cores):
        oc = r.results[c]["out"]
        for k, wid in enumerate(core_wins[c]):
            if wid < 0:
                continue
            nn = min(128, N - wid * 128)
            out[wid * 128:wid * 128 + nn, :] = oc[k * 128:k * 128 + nn, :]
    return out, r


# ----------------------------------------------------------------------------
# Self-contained harness entry point: full inputs in, full output out.
# ----------------------------------------------------------------------------
import os as _os


def kernel(x, edge_index, W, att_src, att_dst, bias):
    x = np.asarray(x, np.float32)
    edge_index = np.asarray(edge_index)
    W = np.asarray(W, np.float32)
    att_src = np.asarray(att_src, np.float32)
    att_dst = np.asarray(att_dst, np.float32)
    bias = np.asarray(bias, np.float32)
    trace = _os.environ.get("GAT_TRACE", "0") == "1"
    out, r = run(x, edge_index, W, att_src, att_dst, bias, n_cores=8, trace=trace)
    if trace and r.exec_time_ns is not None:
        print(f"HW exec time: {r.exec_time_ns} ns")
    return np.ascontiguousarray(out.astype(np.float32))


# revision 3
# speedup vs baseline: 1.4204x; 1.0833x over previous
"""GAT layer on 8 TRN2 cores — V4: gather x-rows (256 B) in transpose mode,
recompute h per edge on TensorE; self-loops handled without gather.

Design:
  - dst windows (128 output nodes each, 391 global) bin-packed onto 8 cores
    by per-window tile count so the SPMD program shape matches all cores.
  - Per non-self edge, gather the 256 B bf16 x-row of its src node from a
    host-staged [N, 128] table via gpsimd dma_gather(transpose=True): output
    lands as [128 c, slots e] — directly the lhsT layout for the h matmul.
  - int16 gather indices use a mid-table base (row 32768) so negative
    indices cover rows [0, 32768) and positives [32768, 50000): one stream.
  - Per 128-edge tile: ps_he = xe_tile^T @ Wext ([128 e, 264] PSUM: 256 h,
    4 a_src-logit, 4 a_dst-logit-weight cols); a_dst of the edge's target
    is accumulated into cols 256:260 by a second (one-hot ohF) matmul;
    e = exp(leakyrelu(a_src + a_dst)); msg = h * e; one-hot ohT matmul
    segment-sums msg + denominators into PSUM.
  - The reference's appended self-loops are excluded from the edge list;
    their contribution (ex_self, ex_self*h_own) is computed from an
    SBUF-resident per-window h table (built once from xoT) and added at
    normalize time. Then divide by denominators, add bias.
  - Critical path = gather descriptor-gen on GpSimd (~9.3 ns/idx); all
    PE/Vector/Scalar work hides under it. Gathers start at t~=0.
"""
import sys
sys.path.insert(0, '/opt/trn_rl_repo')
import numpy as np
import ml_dtypes

import bass_rust as _br
import concourse.bacc as bacc
import concourse.mybir as mybir
import concourse.tile as tile
from concourse import bass_utils

BF16 = ml_dtypes.bfloat16
FP8 = ml_dtypes.float8_e4m3

C_IN = 128
C_OUT_TOT = 256   # HEADS * OUT_CH
HEADS = 4
HC = 64
NEG_SLOPE = 0.2
MID = 32768       # gather index base row
G = 4             # windows per gather call


def host_prep(x, edge_index, W, att_src, att_dst, bias, n_cores=8):
    """Shard + schedule. Returns (cfg, in_maps, core_wins)."""
    N = x.shape[0]
    src = np.asarray(edge_index[0], np.int64).astype(np.int32)
    dst = np.asarray(edge_index[1], np.int64).astype(np.int32)

    NWG = (N + 127) // 128                     # global windows
    NW = (NWG + n_cores - 1) // n_cores        # window slots per core

    order = np.argsort(dst, kind='stable')
    src_s, dst_s = src[order], dst[order]
    w_lo = np.searchsorted(dst_s, np.arange(NWG) * 128, 'left')
    w_hi = np.searchsorted(dst_s, (np.arange(NWG) + 1) * 128, 'left')
    w_cnt = w_hi - w_lo
    w_tiles = np.maximum((w_cnt + 127) // 128, 1)

    # bin-pack windows onto cores: biggest-first to least-loaded core
    core_wins = [[] for _ in range(n_cores)]
    core_load = np.zeros(n_cores, np.int64)
    for wid in np.argsort(-w_tiles, kind='stable'):
        c = int(np.argmin(core_load))
        core_wins[c].append(int(wid))
        core_load[c] += w_tiles[wid]

    # per-slot tile count: max across cores (SPMD shape)
    T_slot = np.ones(NW, np.int64)
    for c in range(n_cores):
        real = sorted(core_wins[c], key=lambda w: -w_tiles[w])
        core_wins[c] = real + [-1] * (NW - len(real))
        for k, w in enumerate(core_wins[c]):
            if w >= 0:
                T_slot[k] = max(T_slot[k], w_tiles[w])

    NCALL = (NW + G - 1) // G
    call_slots = []
    for k in range(NCALL):
        call_slots.append(int(T_slot[k * G:(k + 1) * G].sum()) * 128)
    TOT = sum(call_slots)

    cfg = dict(N=N, n_cores=n_cores, NW=NW, NCALL=NCALL,
               T_slot=[int(t) for t in T_slot], call_slots=call_slots,
               TOT=TOT)

    xT16 = np.ascontiguousarray(np.asarray(x, np.float32)).astype(BF16)  # [N,128]
    W_b = np.asarray(W, np.float32).astype(BF16)           # [128, 256]
    WT_b = np.ascontiguousarray(np.asarray(W).T).astype(BF16)  # [256, 128]
    att_flatT = np.zeros((C_OUT_TOT, 2 * HEADS), np.float32)
    for h in range(HEADS):
        att_flatT[h * HC:(h + 1) * HC, h] = np.asarray(att_src)[h]
        att_flatT[h * HC:(h + 1) * HC, HEADS + h] = np.asarray(att_dst)[h]
    att_flatT_b = att_flatT.astype(BF16)                   # [256, 8]
    bias_bc = np.broadcast_to(np.asarray(bias, np.float32), (128, C_OUT_TOT)).copy()
    xTfull = xT16.T                                        # [128, N] view

    in_maps = []
    for c in range(n_cores):
        idx = np.zeros(TOT, np.int16)
        ohT = np.zeros((128, TOT), FP8)
        ohF = np.zeros((128, TOT), FP8)
        xoT = np.zeros((128, NW * 128), BF16)
        col = 0
        for k, wid in enumerate(core_wins[c]):
            ts = int(T_slot[k]) * 128
            if wid >= 0:
                nn = min(128, N - wid * 128)
                xoT[:, k * 128:k * 128 + nn] = xTfull[:, wid * 128:wid * 128 + nn]
                sw = src_s[w_lo[wid]:w_hi[wid]].copy()
                dw = dst_s[w_lo[wid]:w_hi[wid]].copy() - wid * 128
                n = len(sw)
                assert n <= ts
                idx[col:col + n] = (sw - MID).astype(np.int16)
                # trailing-negative guard: last element of window block
                if n == ts and n > 0 and sw[n - 1] < MID:
                    pos = np.nonzero(sw >= MID)[0]
                    assert len(pos), "window with all-src<MID and full tiles"
                    p = pos[-1]
                    idx[col + p], idx[col + n - 1] = idx[col + n - 1], idx[col + p]
                    sw[p], sw[n - 1] = sw[n - 1], sw[p]
                    dw[p], dw[n - 1] = dw[n - 1], dw[p]
                e_pos = np.arange(n)
                lanes = e_pos % 128
                tiles = e_pos // 128
                ohT[lanes, col + tiles * 128 + dw] = 1.0
                ohF[dw, col + tiles * 128 + lanes] = 1.0
            col += ts
        assert col == TOT
        wrapped = idx.reshape(TOT // 16, 16).T             # [16, TOT/16]
        idx16 = np.tile(wrapped, (8, 1)).copy()            # [128, TOT/16]
        in_maps.append({
            "xtab": xT16, "xoT": xoT,
            "Wb": W_b, "WTb": WT_b, "attT": att_flatT_b, "bias_bc": bias_bc,
            "idx16": idx16, "ohT": ohT, "ohF": ohF,
        })
    return cfg, in_maps, core_wins


def build_program(cfg):
    N, NW, NCALL, TOT = (cfg[k] for k in ("N", "NW", "NCALL", "TOT"))
    T_slot, call_slots = cfg["T_slot"], cfg["call_slots"]
    n_cores = cfg["n_cores"]
    MAXSLOTS = max(call_slots)
    dt = mybir.dt

    nc = bacc.Bacc("TRN2", target_bir_lowering=False, debug=False,
                   num_devices=n_cores)
    t_xtab = nc.dram_tensor("xtab", (N, C_IN), dt.bfloat16, kind="ExternalInput")
    t_xoT = nc.dram_tensor("xoT", (C_IN, NW * 128), dt.bfloat16, kind="ExternalInput")
    t_Wb = nc.dram_tensor("Wb", (C_IN, C_OUT_TOT), dt.bfloat16, kind="ExternalInput")
    t_WTb = nc.dram_tensor("WTb", (C_OUT_TOT, C_IN), dt.bfloat16, kind="ExternalInput")
    t_attT = nc.dram_tensor("attT", (C_OUT_TOT, 2 * HEADS), dt.bfloat16, kind="ExternalInput")
    t_bias = nc.dram_tensor("bias_bc", (128, C_OUT_TOT), dt.float32, kind="ExternalInput")
    t_idx = nc.dram_tensor("idx16", (128, TOT // 16), dt.int16, kind="ExternalInput")
    t_ohT = nc.dram_tensor("ohT", (128, TOT), dt.float8e4, kind="ExternalInput")
    t_ohF = nc.dram_tensor("ohF", (128, TOT), dt.float8e4, kind="ExternalInput")
    t_out = nc.dram_tensor("out", (NW * 128, C_OUT_TOT), dt.float32, kind="ExternalOutput")

    ap_mid = t_xtab.ap()[MID:N, :]
    EXT = C_OUT_TOT + 2 * HEADS      # 264

    with tile.TileContext(nc) as tc:
        with tc.tile_pool(name="const", bufs=1) as cpool:
            Wext_sb = cpool.tile([C_IN, EXT], dt.bfloat16)
            nc.sync.dma_start(out=Wext_sb[:, 0:C_OUT_TOT], in_=t_Wb.ap())
            bias_sb = cpool.tile([128, C_OUT_TOT], dt.float32)
            nc.sync.dma_start(out=bias_sb, in_=t_bias.ap())
            hself = cpool.tile([128, NW, EXT], dt.bfloat16)

            # w_att = W @ att_flatT -> Wext cols 256:264
            with tc.tile_pool(name="watt_ps", bufs=1, space="PSUM") as wpp, \
                 tc.tile_pool(name="watt_sb", bufs=1) as wsp:
                ps_watt = wpp.tile([C_IN, 2 * HEADS], dt.float32)
                wt0 = wsp.tile([128, C_IN], dt.bfloat16)
                wt1 = wsp.tile([128, C_IN], dt.bfloat16)
                at0 = wsp.tile([128, 2 * HEADS], dt.bfloat16)
                at1 = wsp.tile([128, 2 * HEADS], dt.bfloat16)
                nc.sync.dma_start(out=wt0, in_=t_WTb.ap()[0:128, :])
                nc.sync.dma_start(out=wt1, in_=t_WTb.ap()[128:256, :])
                nc.sync.dma_start(out=at0, in_=t_attT.ap()[0:128, :])
                nc.sync.dma_start(out=at1, in_=t_attT.ap()[128:256, :])
                nc.tensor.matmul(out=ps_watt, lhsT=wt0, rhs=at0, start=True, stop=False)
                nc.tensor.matmul(out=ps_watt, lhsT=wt1, rhs=at1, start=False, stop=True)
                nc.vector.tensor_copy(out=Wext_sb[:, C_OUT_TOT:EXT], in_=ps_watt)

            # per-window own-node h/logit table (for a_dst + self-loop path)
            with tc.tile_pool(name="adx", bufs=2) as adx, \
                 tc.tile_pool(name="adps", bufs=2, space="PSUM") as adps:
                for w in range(NW):
                    xo = adx.tile([128, 128], dt.bfloat16, tag="xo")
                    nc.sync.dma_start(out=xo, in_=t_xoT.ap()[:, w * 128:(w + 1) * 128])
                    ps_o = adps.tile([128, EXT], dt.float32, tag="ps_o")
                    nc.tensor.matmul(out=ps_o, lhsT=xo, rhs=Wext_sb,
                                     start=True, stop=True)
                    nc.scalar.copy(out=hself[:, w, :], in_=ps_o)

            # ---------- main: gather + per-tile pipeline ----------
            with tc.tile_pool(name="xep", bufs=3) as xep, \
                 tc.tile_pool(name="idxp", bufs=3) as idxp, \
                 tc.tile_pool(name="ohp", bufs=2) as ohp, \
                 tc.tile_pool(name="msgp", bufs=3) as msgp, \
                 tc.tile_pool(name="sp", bufs=4) as sp, \
                 tc.tile_pool(name="hps", bufs=5, space="PSUM") as hps, \
                 tc.tile_pool(name="wps", bufs=2, space="PSUM") as wps:
                col0 = 0          # global slot offset of current call
                for k in range(NCALL):
                    cs = call_slots[k]
                    idxc = idxp.tile([128, MAXSLOTS // 16], dt.int16, tag="idxc")
                    nc.sync.dma_start(out=idxc[:, 0:cs // 16],
                                      in_=t_idx.ap()[:, col0 // 16:(col0 + cs) // 16])
                    xe = xep.tile([128, 1, MAXSLOTS], dt.bfloat16, tag="xe")
                    nc.gpsimd.dma_gather(
                        out_ap=xe[:, :, 0:cs], in_ap=ap_mid,
                        idxs_ap=idxc[:, 0:cs // 16],
                        num_idxs=cs, num_idxs_reg=cs, elem_size=C_IN,
                        transpose=True, single_packet=False,
                    )
                    ohT_b = ohp.tile([128, MAXSLOTS], dt.float8e4, tag="ohT")
                    ohF_b = ohp.tile([128, MAXSLOTS], dt.float8e4, tag="ohF")
                    for q in range(4):
                        q0, q1 = q * cs // 4 // 128 * 128, (q + 1) * cs // 4 // 128 * 128
                        if q == 3:
                            q1 = cs
                        nc.sync.dma_start(out=ohT_b[:, q0:q1],
                                          in_=t_ohT.ap()[:, col0 + q0:col0 + q1])
                        nc.sync.dma_start(out=ohF_b[:, q0:q1],
                                          in_=t_ohF.ap()[:, col0 + q0:col0 + q1])

                    ccol = 0      # call-local slot offset
                    for w in range(k * G, min((k + 1) * G, NW)):
                        T = T_slot[w]
                        msg = msgp.tile([128, T, EXT], dt.bfloat16, tag="msg",
                                        name=f"msg{T}")
                        for t in range(T):
                            ps_he = hps.tile([128, EXT], dt.float32, tag="ps_he")
                            nc.tensor.matmul(
                                out=ps_he,
                                lhsT=xe[:, 0, ccol + t * 128:ccol + (t + 1) * 128],
                                rhs=Wext_sb, start=True, stop=False,
                                skip_group_check=True)
                            # accumulate a_dst[dst(e)] into the a_src logit cols
                            nc.tensor.matmul(
                                out=ps_he[:, 256:260],
                                lhsT=ohF_b[:, ccol + t * 128:ccol + (t + 1) * 128],
                                rhs=hself[:, w, 260:264],
                                start=False, stop=True, skip_group_check=True)
                            e2 = sp.tile([128, HEADS], dt.float32, tag="e2")
                            nc.scalar.activation(out=e2, in_=ps_he[:, 256:260],
                                                 func=mybir.ActivationFunctionType.Prelu,
                                                 alpha=NEG_SLOPE)
                            nc.scalar.activation(out=msg[:, t, 256:260], in_=e2,
                                                 func=mybir.ActivationFunctionType.Exp)
                            exb = msg[:, t, 256:260].unsqueeze(2).broadcast_to(
                                [128, HEADS, HC])
                            nc.vector.tensor_tensor(
                                out=msg[:, t, 0:C_OUT_TOT].rearrange(
                                    "p (h c) -> p h c", h=HEADS),
                                in0=ps_he[:, 0:C_OUT_TOT].rearrange(
                                    "p (h c) -> p h c", h=HEADS),
                                in1=exb, op=mybir.AluOpType.mult)

                        ps_win = wps.tile([128, 260], dt.float32, tag="ps_win")
                        for t in range(T):
                            nc.tensor.matmul(
                                out=ps_win,
                                lhsT=ohT_b[:, ccol + t * 128:ccol + (t + 1) * 128],
                                rhs=msg[:, t, 0:260],
                                start=(t == 0), stop=(t == T - 1))

                        # self-loop contribution + normalize + bias
                        zs = sp.tile([128, HEADS], dt.float32, tag="zs")
                        nc.vector.tensor_tensor(out=zs, in0=hself[:, w, 256:260],
                                                in1=hself[:, w, 260:264],
                                                op=mybir.AluOpType.add)
                        zp = sp.tile([128, HEADS], dt.float32, tag="zp")
                        nc.scalar.activation(out=zp, in_=zs,
                                             func=mybir.ActivationFunctionType.Prelu,
                                             alpha=NEG_SLOPE)
                        exs = sp.tile([128, HEADS], dt.bfloat16, tag="exs")
                        nc.scalar.activation(out=exs, in_=zp,
                                             func=mybir.ActivationFunctionType.Exp)
                        msgs = sp.tile([128, C_OUT_TOT], dt.float32, tag="msgs")
                        exsb = exs.unsqueeze(2).broadcast_to([128, HEADS, HC])
                        nc.vector.tensor_tensor(
                            out=msgs.rearrange("p (h c) -> p h c", h=HEADS),
                            in0=hself[:, w, 0:C_OUT_TOT].rearrange(
                                "p (h c) -> p h c", h=HEADS),
                            in1=exsb, op=mybir.AluOpType.mult)
                        den = sp.tile([128, HEADS], dt.float32, tag="den")
                        nc.vector.tensor_tensor(out=den, in0=ps_win[:, 256:260],
                                                in1=exs, op=mybir.AluOpType.add)
                        rcp = sp.tile([128, HEADS], dt.float32, tag="rcp")
                        nc.vector.reciprocal(out=rcp, in_=den)
                        num = sp.tile([128, C_OUT_TOT], dt.float32, tag="num")
                        nc.vector.tensor_tensor(out=num, in0=ps_win[:, 0:C_OUT_TOT],
                                                in1=msgs, op=mybir.AluOpType.add)
                        osb = sp.tile([128, C_OUT_TOT], dt.float32, tag="osb")
                        for h in range(HEADS):
                            nc.vector.tensor_scalar(
                                out=osb[:, h * HC:(h + 1) * HC],
                                in0=num[:, h * HC:(h + 1) * HC],
                                scalar1=rcp[:, h:h + 1], scalar2=None,
                                op0=mybir.AluOpType.mult)
                        nc.vector.tensor_tensor(out=osb, in0=osb, in1=bias_sb,
                                                op=mybir.AluOpType.add)
                        nc.sync.dma_start(out=t_out.ap()[w * 128:(w + 1) * 128, :],
                                          in_=osb)
                        ccol += T * 128
                    col0 += cs

    nc.finalize()
    return nc


def register_ntff_hook():
    import types
    import antenv
    if getattr(antenv, 'axon_hooks', None) is not None:
        return
    mod = types.ModuleType('antenv.axon_hooks')
    _hook = [None]
    mod.set_axon_ntff_profile_hook = lambda h: _hook.__setitem__(0, h)
    mod.get_axon_ntff_profile_hook = lambda: _hook[0]
    sys.modules['antenv.axon_hooks'] = mod
    antenv.axon_hooks = mod
    try:
        from trn_agent_boot.trn_boot import _ntff_profile_via_ctypes
        mod.set_axon_ntff_profile_hook(
            _ntff_profile_via_ctypes('/opt/axon/libaxon_pjrt.so'))
    except Exception:
        pass


def run(x, edge_index, W, att_src, att_dst, bias, n_cores=8, trace=False):
    cfg, in_maps, core_wins = host_prep(x, edge_index, W, att_src, att_dst,
                                        bias, n_cores)
    nc = build_program(cfg)
    if trace:
        register_ntff_hook()
    r = bass_utils.run_bass_kernel_spmd(nc, in_maps,
                                        core_ids=list(range(n_cores)),
                                        trace=trace)
    N = cfg["N"]
    out = np.empty((N, C_OUT_TOT), np.float32)
    for c in range(n_cores):
        oc = r.results[c]["out"]
        for k, wid in enumerate(core_wins[c]):
            if wid < 0:
                continue
            nn = min(128, N - wid * 128)
            out[wid * 128:wid * 128 + nn, :] = oc[k * 128:k * 128 + nn, :]
    return out, r


# ----------------------------------------------------------------------------
# Self-contained harness entry point: full inputs in, full output out.
# ----------------------------------------------------------------------------
import os as _os


def kernel(x, edge_index, W, att_src, att_dst, bias):
    x = np.asarray(x, np.float32)
    edge_index = np.asarray(edge_index)
    W = np.asarray(W, np.float32)
    att_src = np.asarray(att_src, np.float32)
    att_dst = np.asarray(att_dst, np.float32)
    bias = np.asarray(bias, np.float32)
    trace = _os.environ.get("GAT_TRACE", "0") == "1"
    out, r = run(x, edge_index, W, att_src, att_dst, bias, n_cores=8, trace=trace)
    if trace and r.exec_time_ns is not None:
        print(f"HW exec time: {r.exec_time_ns} ns")
    return np.ascontiguousarray(out.astype(np.float32))


# revision 8
# speedup vs baseline: 1.6424x; 1.1563x over previous
"""GAT layer on 8 TRN2 cores — V4: gather x-rows (256 B) in transpose mode,
recompute h per edge on TensorE; self-loops handled without gather.

Design:
  - dst windows (128 output nodes each, 391 global) bin-packed onto 8 cores
    by per-window tile count so the SPMD program shape matches all cores.
  - Per non-self edge, gather the 256 B bf16 x-row of its src node from a
    host-staged [N, 128] table via gpsimd dma_gather(transpose=True): output
    lands as [128 c, slots e] — directly the lhsT layout for the h matmul.
  - int16 gather indices use a mid-table base (row 32768) so negative
    indices cover rows [0, 32768) and positives [32768, 50000): one stream.
  - Per 128-edge tile: ps_he = xe_tile^T @ Wext ([128 e, 264] PSUM: 256 h,
    4 a_src-logit, 4 a_dst-logit-weight cols); a_dst of the edge's target
    is accumulated into cols 256:260 by a second (one-hot ohF) matmul;
    e = exp(leakyrelu(a_src + a_dst)); msg = h * e; one-hot ohT matmul
    segment-sums msg + denominators into PSUM.
  - The reference's appended self-loops are excluded from the edge list;
    their contribution (ex_self, ex_self*h_own) is computed from an
    SBUF-resident per-window h table (built once from xoT) and added at
    normalize time. Then divide by denominators, add bias.
  - Critical path = gather descriptor-gen on GpSimd (~9.3 ns/idx); all
    PE/Vector/Scalar work hides under it. Gathers start at t~=0.
"""
import sys
sys.path.insert(0, '/opt/trn_rl_repo')
import numpy as np
import ml_dtypes

import bass_rust as _br
import concourse.bacc as bacc
import concourse.mybir as mybir
import concourse.tile as tile
from concourse import bass_utils

BF16 = ml_dtypes.bfloat16
FP8 = ml_dtypes.float8_e4m3

C_IN = 128
C_OUT_TOT = 256   # HEADS * OUT_CH
HEADS = 4
HC = 64
NEG_SLOPE = 0.2
MID = 32768       # gather index base row
G = 4             # windows per gather call


def host_prep(x, edge_index, W, att_src, att_dst, bias, n_cores=8):
    """Shard + schedule. Returns (cfg, in_maps, core_wins)."""
    N = x.shape[0]
    src = np.asarray(edge_index[0], np.int64).astype(np.int32)
    dst = np.asarray(edge_index[1], np.int64).astype(np.int32)

    NWG = (N + 127) // 128                     # global windows
    NW = (NWG + n_cores - 1) // n_cores        # window slots per core

    order = np.argsort(dst, kind='stable')
    src_s, dst_s = src[order], dst[order]
    w_lo = np.searchsorted(dst_s, np.arange(NWG) * 128, 'left')
    w_hi = np.searchsorted(dst_s, (np.arange(NWG) + 1) * 128, 'left')
    w_cnt = w_hi - w_lo
    w_tiles = np.maximum((w_cnt + 127) // 128, 1)

    # bin-pack windows onto cores: biggest-first to least-loaded core
    core_wins = [[] for _ in range(n_cores)]
    core_load = np.zeros(n_cores, np.int64)
    for wid in np.argsort(-w_tiles, kind='stable'):
        c = int(np.argmin(core_load))
        core_wins[c].append(int(wid))
        core_load[c] += w_tiles[wid]

    # per-slot tile count: max across cores (SPMD shape)
    T_slot = np.ones(NW, np.int64)
    for c in range(n_cores):
        real = sorted(core_wins[c], key=lambda w: -w_tiles[w])
        core_wins[c] = real + [-1] * (NW - len(real))
        for k, w in enumerate(core_wins[c]):
            if w >= 0:
                T_slot[k] = max(T_slot[k], w_tiles[w])

    NCALL = (NW + G - 1) // G
    call_slots = []
    for k in range(NCALL):
        call_slots.append(int(T_slot[k * G:(k + 1) * G].sum()) * 128)
    TOT = sum(call_slots)

    cfg = dict(N=N, n_cores=n_cores, NW=NW, NCALL=NCALL,
               T_slot=[int(t) for t in T_slot], call_slots=call_slots,
               TOT=TOT)

    xT16 = np.ascontiguousarray(np.asarray(x, np.float32)).astype(BF16)  # [N,128]
    W_b = np.asarray(W, np.float32).astype(BF16)           # [128, 256]
    WT_b = np.ascontiguousarray(np.asarray(W).T).astype(BF16)  # [256, 128]
    att_flatT = np.zeros((C_OUT_TOT, 2 * HEADS), np.float32)
    for h in range(HEADS):
        att_flatT[h * HC:(h + 1) * HC, h] = np.asarray(att_src)[h]
        att_flatT[h * HC:(h + 1) * HC, HEADS + h] = np.asarray(att_dst)[h]
    att_flatT_b = att_flatT.astype(BF16)                   # [256, 8]
    bias_bc = np.broadcast_to(np.asarray(bias, np.float32), (128, C_OUT_TOT)).copy()
    ident8 = np.eye(128, dtype=np.float32).astype(FP8)     # [128, 128]
    xTfull = xT16.T                                        # [128, N] view

    in_maps = []
    for c in range(n_cores):
        idx = np.zeros(TOT, np.int16)
        ohT = np.zeros((128, TOT), FP8)
        ohF = np.zeros((128, TOT), FP8)
        xoT = np.zeros((128, NW * 128), BF16)
        col = 0
        for k, wid in enumerate(core_wins[c]):
            ts = int(T_slot[k]) * 128
            if wid >= 0:
                nn = min(128, N - wid * 128)
                xoT[:, k * 128:k * 128 + nn] = xTfull[:, wid * 128:wid * 128 + nn]
                sw = src_s[w_lo[wid]:w_hi[wid]].copy()
                dw = dst_s[w_lo[wid]:w_hi[wid]].copy() - wid * 128
                n = len(sw)
                assert n <= ts
                idx[col:col + n] = (sw - MID).astype(np.int16)
                # trailing-negative guard: last element of window block
                if n == ts and n > 0 and sw[n - 1] < MID:
                    pos = np.nonzero(sw >= MID)[0]
                    assert len(pos), "window with all-src<MID and full tiles"
                    p = pos[-1]
                    idx[col + p], idx[col + n - 1] = idx[col + n - 1], idx[col + p]
                    sw[p], sw[n - 1] = sw[n - 1], sw[p]
                    dw[p], dw[n - 1] = dw[n - 1], dw[p]
                e_pos = np.arange(n)
                lanes = e_pos % 128
                tiles = e_pos // 128
                ohT[lanes, col + tiles * 128 + dw] = 1.0
                ohF[dw, col + tiles * 128 + lanes] = 1.0
            col += ts
        assert col == TOT
        wrapped = idx.reshape(TOT // 16, 16).T             # [16, TOT/16]
        idx16 = np.tile(wrapped, (8, 1)).copy()            # [128, TOT/16]
        in_maps.append({
            "xtab": xT16, "xoT": xoT,
            "Wb": W_b, "WTb": WT_b, "attT": att_flatT_b, "bias_bc": bias_bc,
            "ident8": ident8, "idx16": idx16, "ohT": ohT, "ohF": ohF,
        })
    return cfg, in_maps, core_wins


def build_program(cfg):
    N, NW, NCALL, TOT = (cfg[k] for k in ("N", "NW", "NCALL", "TOT"))
    T_slot, call_slots = cfg["T_slot"], cfg["call_slots"]
    n_cores = cfg["n_cores"]
    MAXSLOTS = max(call_slots)
    dt = mybir.dt

    nc = bacc.Bacc("TRN2", target_bir_lowering=False, debug=False,
                   num_devices=n_cores)
    t_xtab = nc.dram_tensor("xtab", (N, C_IN), dt.bfloat16, kind="ExternalInput")
    t_xoT = nc.dram_tensor("xoT", (C_IN, NW * 128), dt.bfloat16, kind="ExternalInput")
    t_Wb = nc.dram_tensor("Wb", (C_IN, C_OUT_TOT), dt.bfloat16, kind="ExternalInput")
    t_WTb = nc.dram_tensor("WTb", (C_OUT_TOT, C_IN), dt.bfloat16, kind="ExternalInput")
    t_attT = nc.dram_tensor("attT", (C_OUT_TOT, 2 * HEADS), dt.bfloat16, kind="ExternalInput")
    t_bias = nc.dram_tensor("bias_bc", (128, C_OUT_TOT), dt.float32, kind="ExternalInput")
    t_idx = nc.dram_tensor("idx16", (128, TOT // 16), dt.int16, kind="ExternalInput")
    t_ohT = nc.dram_tensor("ohT", (128, TOT), dt.float8e4, kind="ExternalInput")
    t_ohF = nc.dram_tensor("ohF", (128, TOT), dt.float8e4, kind="ExternalInput")
    t_id8 = nc.dram_tensor("ident8", (128, 128), dt.float8e4, kind="ExternalInput")
    t_out = nc.dram_tensor("out", (NW * 128, C_OUT_TOT), dt.float32, kind="ExternalOutput")

    ap_mid = t_xtab.ap()[MID:N, :]
    EXT = C_OUT_TOT + 2 * HEADS      # 264
    call_off = [0]
    for cs in call_slots:
        call_off.append(call_off[-1] + cs)

    with tile.TileContext(nc) as tc:
        with tc.tile_pool(name="const", bufs=1) as cpool, \
             tc.tile_pool(name="xep", bufs=3) as xep, \
             tc.tile_pool(name="idxp", bufs=3) as idxp, \
             tc.tile_pool(name="ohp", bufs=2) as ohp:

            def issue_call_loads(k):
                cs = call_slots[k]
                c0 = call_off[k]
                idxc = idxp.tile([128, MAXSLOTS // 16], dt.int16, tag="idxc",
                                 name="idxc")
                nc.sync.dma_start(out=idxc[:, 0:cs // 16],
                                  in_=t_idx.ap()[:, c0 // 16:(c0 + cs) // 16])
                xe = xep.tile([128, 1, MAXSLOTS], dt.bfloat16, tag="xe", name="xe")
                nc.gpsimd.dma_gather(
                    out_ap=xe[:, :, 0:cs], in_ap=ap_mid,
                    idxs_ap=idxc[:, 0:cs // 16],
                    num_idxs=cs, num_idxs_reg=cs, elem_size=C_IN,
                    transpose=True, single_packet=False,
                )
                ohT_b = ohp.tile([128, MAXSLOTS], dt.float8e4, tag="ohT",
                                 name="ohT_b")
                ohF_b = ohp.tile([128, MAXSLOTS], dt.float8e4, tag="ohF",
                                 name="ohF_b")
                for q in range(4):
                    q0 = q * cs // 4 // 128 * 128
                    q1 = cs if q == 3 else (q + 1) * cs // 4 // 128 * 128
                    nc.sync.dma_start(out=ohT_b[:, q0:q1],
                                      in_=t_ohT.ap()[:, c0 + q0:c0 + q1])
                    nc.sync.dma_start(out=ohF_b[:, q0:q1],
                                      in_=t_ohF.ap()[:, c0 + q0:c0 + q1])
                return xe, ohT_b, ohF_b

            cur = issue_call_loads(0)     # gathers start at t~=0

            Wext_sb = cpool.tile([C_IN, EXT], dt.bfloat16)
            nc.sync.dma_start(out=Wext_sb[:, 0:C_OUT_TOT], in_=t_Wb.ap())
            bias_sb = cpool.tile([128, C_OUT_TOT], dt.float32)
            nc.sync.dma_start(out=bias_sb, in_=t_bias.ap())
            ident_sb = cpool.tile([128, 128], dt.float8e4)
            nc.sync.dma_start(out=ident_sb, in_=t_id8.ap())
            hself = cpool.tile([128, NW, EXT], dt.bfloat16)

            # w_att = W @ att_flatT -> Wext cols 256:264
            with tc.tile_pool(name="watt_ps", bufs=1, space="PSUM") as wpp, \
                 tc.tile_pool(name="watt_sb", bufs=1) as wsp:
                ps_watt = wpp.tile([C_IN, 2 * HEADS], dt.float32)
                wt0 = wsp.tile([128, C_IN], dt.bfloat16)
                wt1 = wsp.tile([128, C_IN], dt.bfloat16)
                at0 = wsp.tile([128, 2 * HEADS], dt.bfloat16)
                at1 = wsp.tile([128, 2 * HEADS], dt.bfloat16)
                nc.sync.dma_start(out=wt0, in_=t_WTb.ap()[0:128, :])
                nc.sync.dma_start(out=wt1, in_=t_WTb.ap()[128:256, :])
                nc.sync.dma_start(out=at0, in_=t_attT.ap()[0:128, :])
                nc.sync.dma_start(out=at1, in_=t_attT.ap()[128:256, :])
                nc.tensor.matmul(out=ps_watt, lhsT=wt0, rhs=at0, start=True, stop=False)
                nc.tensor.matmul(out=ps_watt, lhsT=wt1, rhs=at1, start=False, stop=True)
                nc.vector.tensor_copy(out=Wext_sb[:, C_OUT_TOT:EXT], in_=ps_watt)

            # per-window own-node h/logit table (for a_dst + self-loop path)
            with tc.tile_pool(name="adx", bufs=2) as adx, \
                 tc.tile_pool(name="adps", bufs=2, space="PSUM") as adps:
                for w in range(NW):
                    xo = adx.tile([128, 128], dt.bfloat16, tag="xo")
                    nc.sync.dma_start(out=xo, in_=t_xoT.ap()[:, w * 128:(w + 1) * 128])
                    ps_o = adps.tile([128, EXT], dt.float32, tag="ps_o")
                    nc.tensor.matmul(out=ps_o, lhsT=xo, rhs=Wext_sb,
                                     start=True, stop=True)
                    nc.scalar.copy(out=hself[:, w, :], in_=ps_o)

            # ---------- main: gather + per-tile pipeline ----------
            with tc.tile_pool(name="msgp", bufs=3) as msgp, \
                 tc.tile_pool(name="sp", bufs=4) as sp, \
                 tc.tile_pool(name="hps", bufs=5, space="PSUM") as hps, \
                 tc.tile_pool(name="wps", bufs=2, space="PSUM") as wps:
                for k in range(NCALL):
                    xe, ohT_b, ohF_b = cur
                    if k + 1 < NCALL:
                        cur = issue_call_loads(k + 1)

                    ccol = 0      # call-local slot offset
                    for w in range(k * G, min((k + 1) * G, NW)):
                        T = T_slot[w]
                        msg = msgp.tile([128, T, EXT], dt.bfloat16, tag="msg",
                                        name=f"msg{T}")
                        for t in range(T):
                            ps_he = hps.tile([128, EXT], dt.float32, tag="ps_he")
                            nc.tensor.matmul(
                                out=ps_he,
                                lhsT=xe[:, 0, ccol + t * 128:ccol + (t + 1) * 128],
                                rhs=Wext_sb, start=True, stop=False,
                                skip_group_check=True)
                            # accumulate a_dst[dst(e)] into the a_src logit cols
                            nc.tensor.matmul(
                                out=ps_he[:, 256:260],
                                lhsT=ohF_b[:, ccol + t * 128:ccol + (t + 1) * 128],
                                rhs=hself[:, w, 260:264],
                                start=False, stop=True, skip_group_check=True)
                            e2 = sp.tile([128, HEADS], dt.float32, tag="e2")
                            nc.scalar.activation(out=e2, in_=ps_he[:, 256:260],
                                                 func=mybir.ActivationFunctionType.Prelu,
                                                 alpha=NEG_SLOPE)
                            nc.scalar.activation(out=msg[:, t, 256:260], in_=e2,
                                                 func=mybir.ActivationFunctionType.Exp)
                            exb = msg[:, t, 256:260].unsqueeze(2).broadcast_to(
                                [128, HEADS, HC])
                            nc.vector.tensor_tensor(
                                out=msg[:, t, 0:C_OUT_TOT].rearrange(
                                    "p (h c) -> p h c", h=HEADS),
                                in0=ps_he[:, 0:C_OUT_TOT].rearrange(
                                    "p (h c) -> p h c", h=HEADS),
                                in1=exb, op=mybir.AluOpType.mult)

                        # self-loop message tile: cols 0:256 = h_own*ex_self,
                        # 256:260 = ex_self (rides the same aggregation matmul)
                        zs = sp.tile([128, HEADS], dt.float32, tag="zs")
                        nc.vector.tensor_tensor(out=zs, in0=hself[:, w, 256:260],
                                                in1=hself[:, w, 260:264],
                                                op=mybir.AluOpType.add)
                        zp = sp.tile([128, HEADS], dt.float32, tag="zp")
                        nc.scalar.activation(out=zp, in_=zs,
                                             func=mybir.ActivationFunctionType.Prelu,
                                             alpha=NEG_SLOPE)
                        msgs = sp.tile([128, 260], dt.bfloat16, tag="msgs")
                        nc.scalar.activation(out=msgs[:, 256:260], in_=zp,
                                             func=mybir.ActivationFunctionType.Exp)
                        exsb = msgs[:, 256:260].unsqueeze(2).broadcast_to(
                            [128, HEADS, HC])
                        nc.vector.tensor_tensor(
                            out=msgs[:, 0:C_OUT_TOT].rearrange(
                                "p (h c) -> p h c", h=HEADS),
                            in0=hself[:, w, 0:C_OUT_TOT].rearrange(
                                "p (h c) -> p h c", h=HEADS),
                            in1=exsb, op=mybir.AluOpType.mult)

                        ps_win = wps.tile([128, 260], dt.float32, tag="ps_win")
                        for t in range(T):
                            nc.tensor.matmul(
                                out=ps_win,
                                lhsT=ohT_b[:, ccol + t * 128:ccol + (t + 1) * 128],
                                rhs=msg[:, t, 0:260],
                                start=(t == 0), stop=False)
                        nc.tensor.matmul(out=ps_win, lhsT=ident_sb, rhs=msgs,
                                         start=False, stop=True)

                        rcp = sp.tile([128, HEADS], dt.float32, tag="rcp")
                        nc.vector.reciprocal(out=rcp, in_=ps_win[:, 256:260])
                        osb = sp.tile([128, C_OUT_TOT], dt.float32, tag="osb")
                        for h in range(HEADS):
                            nc.vector.tensor_scalar(
                                out=osb[:, h * HC:(h + 1) * HC],
                                in0=ps_win[:, h * HC:(h + 1) * HC],
                                scalar1=rcp[:, h:h + 1], scalar2=None,
                                op0=mybir.AluOpType.mult)
                        nc.vector.tensor_tensor(out=osb, in0=osb, in1=bias_sb,
                                                op=mybir.AluOpType.add)
                        nc.sync.dma_start(out=t_out.ap()[w * 128:(w + 1) * 128, :],
                                          in_=osb)
                        ccol += T * 128

    nc.finalize()
    return nc


def register_ntff_hook():
    import types
    import antenv
    if getattr(antenv, 'axon_hooks', None) is not None:
        return
    mod = types.ModuleType('antenv.axon_hooks')
    _hook = [None]
    mod.set_axon_ntff_profile_hook = lambda h: _hook.__setitem__(0, h)
    mod.get_axon_ntff_profile_hook = lambda: _hook[0]
    sys.modules['antenv.axon_hooks'] = mod
    antenv.axon_hooks = mod
    try:
        from trn_agent_boot.trn_boot import _ntff_profile_via_ctypes
        mod.set_axon_ntff_profile_hook(
            _ntff_profile_via_ctypes('/opt/axon/libaxon_pjrt.so'))
    except Exception:
        pass


def run(x, edge_index, W, att_src, att_dst, bias, n_cores=8, trace=False):
    cfg, in_maps, core_wins = host_prep(x, edge_index, W, att_src, att_dst,
                                        bias, n_cores)
    nc = build_program(cfg)
    if trace:
        register_ntff_hook()
    r = bass_utils.run_bass_kernel_spmd(nc, in_maps,
                                        core_ids=list(range(n_cores)),
                                        trace=trace)
    N = cfg["N"]
    out = np.empty((N, C_OUT_TOT), np.float32)
    for c in range(n_cores):
        oc = r.results[c]["out"]
        for k, wid in enumerate(core_wins[c]):
            if wid < 0:
                continue
            nn = min(128, N - wid * 128)
            out[wid * 128:wid * 128 + nn, :] = oc[k * 128:k * 128 + nn, :]
    return out, r


# ----------------------------------------------------------------------------
# Self-contained harness entry point: full inputs in, full output out.
# ----------------------------------------------------------------------------
import os as _os


def kernel(x, edge_index, W, att_src, att_dst, bias):
    x = np.asarray(x, np.float32)
    edge_index = np.asarray(edge_index)
    W = np.asarray(W, np.float32)
    att_src = np.asarray(att_src, np.float32)
    att_dst = np.asarray(att_dst, np.float32)
    bias = np.asarray(bias, np.float32)
    trace = _os.environ.get("GAT_TRACE", "0") == "1"
    out, r = run(x, edge_index, W, att_src, att_dst, bias, n_cores=8, trace=trace)
    if trace and r.exec_time_ns is not None:
        print(f"HW exec time: {r.exec_time_ns} ns")
    return np.ascontiguousarray(out.astype(np.float32))
